# revision 13
# baseline (speedup 1.0000x reference)
"""MultiHeadAttention Trainium2 kernel (B=4, S=2048, D=1024, H=16, causal).

Sharding: 8 cores = batch(4) x head-group(2). Each core computes its batch's
attention for 8 heads (e-slice of 512) plus the partial out-projection for its
d-slice; host sums the two partials per batch and adds bo.

Layouts (per core, all fp32):
  xT   [D=1024, S=2048]   x[b].T              (d on partitions, 8 tiles)
  wqT  [D, 512]           Wq[eslice,:].T      (QT = wqT.T-contracted with xT)
  QT/KT in SBUF as [e=512, s=2048]            (4 partition tiles)
  V in SBUF as [s, 8 heads, 65]               (64 v-cols + ones col for denom)
  attention in "scores-transposed" orientation: ST[k, q] = (K Q^T)/8,
  ET = exp(ST) masked, out_aug[hd(+den), q] = V_aug^T-contracted with ET,
  normalize by reciprocal of the denominator row, out-proj from [d, s].
"""

import numpy as np

B, S, D, H = 4, 2048, 1024, 16
HD = D // H  # 64
NCORES = 8
HPG = 8          # heads per group (per core)
EP = HPG * HD    # 512, e-slice width per core
SCH = 512        # s-chunk width (q chunks, projection n chunks)
NSC = S // SCH   # 4
NST = S // 128   # 16 s-tiles
NDT = D // 128   # 8 d-tiles
NET = EP // 128  # 4 e-tiles per core

_cache = {}


def _build_program(with_pad, with_bias=False, use_bf16=True, repeat=1):
    import concourse.tile as tile
    from concourse import bacc, mybir

    f32 = mybir.dt.float32
    Exp = mybir.ActivationFunctionType.Exp

    # mdt: dtype for all matmul operands. bf16 runs the PE at 1 cycle/row
    # (fp32r's single-pass mode does not engage on hardware, leaving fp32
    # at 4 cycles/row); accumulation stays fp32 in PSUM.
    mdt = mybir.dt.bfloat16 if use_bf16 else mybir.dt.float32r

    def mm(out_ap, lhsT, rhs, **kw):
        nc.tensor.matmul(out_ap, lhsT, rhs, **kw)

    nc = bacc.Bacc("TRN2", target_bir_lowering=False)

    xT = nc.dram_tensor("xT", (D, S), mdt, kind="ExternalInput")
    wqT = nc.dram_tensor("wqT", (D, EP), mdt, kind="ExternalInput")
    wkT = nc.dram_tensor("wkT", (D, EP), mdt, kind="ExternalInput")
    wvT = nc.dram_tensor("wvT", (D, EP), mdt, kind="ExternalInput")
    woT = nc.dram_tensor("woT", (EP, D), mdt, kind="ExternalInput")
    if with_bias:
        bq = nc.dram_tensor("bq", (EP,), f32, kind="ExternalInput")
        bk = nc.dram_tensor("bk", (EP,), f32, kind="ExternalInput")
        bv = nc.dram_tensor("bv", (EP,), f32, kind="ExternalInput")
    cmask = nc.dram_tensor("cmask", (5, 128, SCH), mdt, kind="ExternalInput")
    if with_pad:
        padm = nc.dram_tensor("padm", (NST, 128), f32, kind="ExternalInput")
    out = nc.dram_tensor("out", (S, D), f32, kind="ExternalOutput")

    xT_t = xT.rearrange("(dt p) s -> p dt s", p=128)
    wqT_t = wqT.rearrange("(dt p) e -> p dt e", p=128)
    wkT_t = wkT.rearrange("(dt p) e -> p dt e", p=128)
    wvT_t = wvT.rearrange("(dt p) e -> p dt e", p=128)
    woT_t = woT.rearrange("(dt p) e -> p dt e", p=128)

    with tile.TileContext(nc) as tc:
        with tc.tile_pool(name="persist", bufs=1) as pp:
            # persistent SBUF tensors (live across phases)
            QT = pp.tile([128, NET, S], mdt)    # [e-tile, s]
            KT = pp.tile([128, NET, S], mdt)
            V = pp.tile([128, NST, HPG, HD + 1], mdt)  # ones col at index HD
            cm = pp.tile([128, 5, SCH], mdt)
            if with_bias:
                bq_sb = pp.tile([128, NET], f32)
                bk_sb = pp.tile([128, NET], f32)
                bv_sb = pp.tile([64, HPG], f32)
            if with_pad:
                pad_sb = pp.tile([128, NST], f32)
                nc.sync.dma_start(out=pad_sb, in_=padm.rearrange("t p -> p t"))

            for p4 in range(5):
                nc.sync.dma_start(out=cm[:, p4, :], in_=cmask[p4])
            if with_bias:
                nc.sync.dma_start(
                    out=bq_sb, in_=bq.rearrange("(t p) -> p t", p=128))
                nc.sync.dma_start(
                    out=bk_sb, in_=bk.rearrange("(t p) -> p t", p=128))
                nc.sync.dma_start(
                    out=bv_sb, in_=bv.rearrange("(h p) -> p h", p=64))
            for st in range(NST):
                if use_bf16:
                    nc.vector.memset(V[:, st, :, HD:HD + 1], 1.0)
                else:
                    nc.vector.memset(V[:, st, :, HD:HD + 1].bitcast(f32), 1.0)

            for _rep in range(repeat):
                # ---------------- Phase 1: projections ----------------
                with tc.tile_pool(name="ph1", bufs=1) as wp, \
                     tc.tile_pool(name="ph1x", bufs=2) as xp, \
                     tc.tile_pool(name="ps1", bufs=4, space="PSUM") as ps1:
                    wq_sb = wp.tile([128, NDT, EP], mdt)
                    wk_sb = wp.tile([128, NDT, EP], mdt)
                    wv_sb = wp.tile([128, NDT, EP], mdt)
                    # per-d-chunk DMAs: the first projection matmuls only
                    # wait on their own 256KB slice, not the full 2MB
                    for dt in range(NDT):
                        nc.sync.dma_start(
                            out=wq_sb[:, dt, :], in_=wqT_t[:, dt, :])
                        nc.sync.dma_start(
                            out=wk_sb[:, dt, :], in_=wkT_t[:, dt, :])
                        nc.sync.dma_start(
                            out=wv_sb[:, dt, :], in_=wvT_t[:, dt, :])

                    for sc in range(NSC):
                        xs = xp.tile([128, NDT, SCH], mdt, tag="xs")
                        for dt in range(NDT):
                            nc.sync.dma_start(
                                out=xs[:, dt, :],
                                in_=xT_t[:, dt, sc * SCH:(sc + 1) * SCH])
                        # QT / KT e-tiles: out[e128, s512] = sum_d wT[d,e] x^T[d,s]
                        for et in range(NET):
                            psq = ps1.tile([128, SCH], f32, tag="psp")
                            for dt in range(NDT):
                                mm(
                                    psq,
                                    wq_sb[:, dt, et * 128:(et + 1) * 128],
                                    xs[:, dt, :],
                                    start=(dt == 0), stop=(dt == NDT - 1))
                            if with_bias:
                                nc.vector.tensor_scalar_add(
                                    QT[:, et, sc * SCH:(sc + 1) * SCH], psq,
                                    bq_sb[:, et:et + 1])
                            else:
                                nc.scalar.copy(
                                    QT[:, et, sc * SCH:(sc + 1) * SCH], psq)
                            psk = ps1.tile([128, SCH], f32, tag="psp")
                            for dt in range(NDT):
                                mm(
                                    psk,
                                    wk_sb[:, dt, et * 128:(et + 1) * 128],
                                    xs[:, dt, :],
                                    start=(dt == 0), stop=(dt == NDT - 1))
                            if with_bias:
                                nc.vector.tensor_scalar_add(
                                    KT[:, et, sc * SCH:(sc + 1) * SCH], psk,
                                    bk_sb[:, et:et + 1])
                            else:
                                nc.vector.tensor_copy(
                                    KT[:, et, sc * SCH:(sc + 1) * SCH], psk)
                        # V s-tiles: out[s128, e512] = sum_d x^T[d,s] wvT[d,e]
                        for st4 in range(4):
                            st = sc * 4 + st4
                            psv = ps1.tile([128, SCH], f32, tag="psp")
                            for dt in range(NDT):
                                mm(
                                    psv,
                                    xs[:, dt, st4 * 128:(st4 + 1) * 128],
                                    wv_sb[:, dt, :],
                                    start=(dt == 0), stop=(dt == NDT - 1))
                            nc.scalar.copy(
                                V[:, st, :, 0:HD],
                                psv.rearrange("p (h v) -> p h v", h=HPG))

                # ---------------- Phase 2+3 ----------------
                with tc.tile_pool(name="ph2", bufs=1) as lp, \
                     tc.tile_pool(name="et", bufs=4) as etp, \
                     tc.tile_pool(name="tmp", bufs=4) as tmpp, \
                     tc.tile_pool(name="fo", bufs=4) as fop, \
                     tc.tile_pool(name="ps_s", bufs=3, space="PSUM") as pss, \
                     tc.tile_pool(name="ps_o", bufs=2, space="PSUM") as pso:
                    OT = lp.tile([128, NET, S], mdt)   # attn out, [d, s] packed
                    wo_sb = lp.tile([128, NET, D], mdt)
                    nc.sync.dma_start(out=wo_sb, in_=woT_t)

                    for h in range(HPG):
                        eb = (h % 2) * 64            # partition base within e-tile
                        et_i = h // 2                # e-tile index for this head
                        for qc in range(NSC):
                            nkt = 4 * (qc + 1)       # causal: k-tiles 0..nkt-1
                            nfull = nkt - 4          # fully-valid k-tiles
                            ps_o = pso.tile([128, SCH], f32, tag="pso")
                            # below-diagonal k-tiles, unmasked, in pairs
                            for kp in range(nfull // 2):
                                ps_s = pss.tile([128, 2, SCH], f32, tag="pss")
                                for j in range(2):
                                    kt = 2 * kp + j
                                    mm(
                                        ps_s[:, j, :],
                                        KT[eb:eb + 64, et_i,
                                           kt * 128:(kt + 1) * 128],
                                        QT[eb:eb + 64, et_i,
                                           qc * SCH:(qc + 1) * SCH],
                                        start=True, stop=True)
                                et_t = etp.tile([128, 2, SCH], mdt, tag="et")
                                nc.scalar.activation(et_t, ps_s, Exp, scale=0.125)
                                for j in range(2):
                                    kt = 2 * kp + j
                                    if with_pad:
                                        nc.vector.tensor_scalar_mul(
                                            et_t[:, j, :], et_t[:, j, :],
                                            pad_sb[:, kt:kt + 1])
                                    mm(
                                        ps_o[0:HD + 1, :],
                                        V[:, kt, h, :],
                                        et_t[:, j, :],
                                        start=(kt == 0), stop=False,
                                        skip_group_check=True)
                            # diagonal 512x512 block: 256-wide q-subchunks
                            # (N=256 keeps fp32r at full rate); only the last
                            # two k-tiles of each subchunk need mask-muls,
                            # at half width ([128,256])
                            for jq in range(2):
                                q0 = qc * SCH + jq * 256
                                nk = 2 * (jq + 1)    # k-tiles of this sub
                                ps_d = pss.tile([128, 2, SCH], f32, tag="pss")
                                for kk in range(nk):
                                    kt = nfull + kk
                                    mm(
                                        ps_d[:, kk // 2,
                                             (kk % 2) * 256:(kk % 2 + 1) * 256],
                                        KT[eb:eb + 64, et_i,
                                           kt * 128:(kt + 1) * 128],
                                        QT[eb:eb + 64, et_i, q0:q0 + 256],
                                        start=True, stop=True)
                                et_d = etp.tile([128, 2, SCH], mdt, tag="et")
                                if nk == 2:
                                    nc.scalar.activation(
                                        et_d[:, 0, :], ps_d[:, 0, :],
                                        Exp, scale=0.125)
                                else:
                                    nc.scalar.activation(
                                        et_d, ps_d, Exp, scale=0.125)
                                # the two partial k-tiles are contiguous in
                                # et_d: one [128,512] mul with the combined
                                # pattern (cmask[4])
                                nc.vector.tensor_mul(
                                    et_d[:, nk // 2 - 1, :],
                                    et_d[:, nk // 2 - 1, :],
                                    cm[:, 4, :])
                                for kk in range(nk):
                                    kt = nfull + kk
                                    if with_pad:
                                        nc.vector.tensor_scalar_mul(
                                            et_d[:, kk // 2,
                                                 (kk % 2) * 256:
                                                 (kk % 2 + 1) * 256],
                                            et_d[:, kk // 2,
                                                 (kk % 2) * 256:
                                                 (kk % 2 + 1) * 256],
                                            pad_sb[:, kt:kt + 1])
                                    mm(
                                        ps_o[0:HD + 1,
                                             jq * 256:(jq + 1) * 256],
                                        V[:, kt, h, :],
                                        et_d[:, kk // 2,
                                             (kk % 2) * 256:(kk % 2 + 1) * 256],
                                        start=(nfull == 0 and kk == 0),
                                        stop=(kk == nk - 1),
                                        skip_group_check=True)
                            # normalize: rec = 1/denominator row (at
                            # partition 0), broadcast on the Pool engine,
                            # multiply straight into the packed OT slot
                            rec = tmpp.tile([1, SCH], f32, tag="rec")
                            nc.vector.reciprocal(rec, ps_o[64:65, :])
                            bc_sb = tmpp.tile([64, SCH], f32, tag="bcs")
                            nc.gpsimd.partition_broadcast(bc_sb, rec)
                            ot_slot = OT[eb:eb + 64, et_i,
                                         qc * SCH:(qc + 1) * SCH]
                            nc.vector.tensor_mul(
                                ot_slot, ps_o[0:HD, :], bc_sb)
                            if with_bias:
                                nc.vector.tensor_scalar_add(
                                    ot_slot, ot_slot, bv_sb[:, h:h + 1])

                    # ---------------- Phase 3: out projection ----------------
                    for st in range(NST):
                        for ec in range(D // SCH):
                            ps_f = pso.tile([128, SCH], f32, tag="pso")
                            for dt in range(NET):
                                mm(
                                    ps_f,
                                    OT[:, dt, st * 128:(st + 1) * 128],
                                    wo_sb[:, dt, ec * SCH:(ec + 1) * SCH],
                                    start=(dt == 0), stop=(dt == NET - 1))
                            fo = fop.tile([128, SCH], f32, tag="fo")
                            (nc.scalar.copy if (st + ec) % 2 == 0
                             else nc.vector.tensor_copy)(fo, ps_f)
                            nc.sync.dma_start(
                                out=out[st * 128:(st + 1) * 128,
                                        ec * SCH:(ec + 1) * SCH],
                                in_=fo)
    nc.compile()
    return nc


def prep_in_maps(inputs, use_bf16=True):
    import ml_dtypes

    mdt_np = ml_dtypes.bfloat16 if use_bf16 else np.float32
    x = np.asarray(inputs["x"], dtype=np.float32)
    mask = np.asarray(inputs["attention_mask"])
    Wq = np.asarray(inputs["Wq"], dtype=np.float32)
    Wk = np.asarray(inputs["Wk"], dtype=np.float32)
    Wv = np.asarray(inputs["Wv"], dtype=np.float32)
    Wo = np.asarray(inputs["Wo"], dtype=np.float32)
    bq = np.asarray(inputs["bq"], dtype=np.float32)
    bk = np.asarray(inputs["bk"], dtype=np.float32)
    bv = np.asarray(inputs["bv"], dtype=np.float32)
    with_pad = not bool((mask != 0).all())
    with_bias = bool(bq.any() or bk.any() or bv.any())

    # causal mask patterns for the 4 diagonal k-tiles of each q-chunk:
    # cmask[p][kk, qq] = 1 if qq >= kk + 128*p  (ST orientation [k, q])
    kk = np.arange(128)[:, None]
    qq = np.arange(SCH)[None, :]
    cmask = [(qq >= kk + 128 * p).astype(np.float32) for p in range(4)]
    cmask.append(np.concatenate(
        [cmask[0][:, 0:256], cmask[1][:, 0:256]], axis=1))
    cmask = np.ascontiguousarray(np.stack(cmask)).astype(mdt_np)

    in_maps = []
    for c in range(NCORES):
        b, g = divmod(c, 2)
        es = slice(g * EP, (g + 1) * EP)
        m = {
            "xT": np.ascontiguousarray(x[b].T).astype(mdt_np),
            "wqT": np.ascontiguousarray(Wq[es, :].T).astype(mdt_np),
            "wkT": np.ascontiguousarray(Wk[es, :].T).astype(mdt_np),
            "wvT": np.ascontiguousarray(Wv[es, :].T).astype(mdt_np),
            "woT": np.ascontiguousarray(Wo[:, es].T).astype(mdt_np),
            "cmask": cmask,
        }
        if with_bias:
            m["bq"] = np.ascontiguousarray(bq[es])
            m["bk"] = np.ascontiguousarray(bk[es])
            m["bv"] = np.ascontiguousarray(bv[es])
        if with_pad:
            m["padm"] = np.ascontiguousarray(
                mask[b].astype(np.float32).reshape(NST, 128))
        in_maps.append(m)
    return in_maps, (with_pad, with_bias)


def kernel(**inputs):
    from concourse import bass_utils

    in_maps, (with_pad, with_bias) = prep_in_maps(inputs)
    bo = np.asarray(inputs["bo"], dtype=np.float32)

    key = ("prog", with_pad, with_bias)
    if key not in _cache:
        _cache[key] = _build_program(with_pad, with_bias)
    nc = _cache[key]

    res = bass_utils.run_bass_kernel_spmd(nc, in_maps, core_ids=list(range(NCORES)))

    final = np.empty((B, S, D), dtype=np.float32)
    for b in range(B):
        final[b] = res.results[2 * b]["out"] + res.results[2 * b + 1]["out"] + bo
    return final

